# revision 14
# baseline (speedup 1.0000x reference)
"""BCEWithLogitsLoss(mean) over (8192, 8192) logits with binary-step targets,
data-parallel over 8 NeuronCores (1024 rows each).

loss = mean(softplus(x) - x * t),  t[i,j] = 1 if j < targets[i] else 0
     = [ sum softplus(x)  -  sum_{j<t_i} x[i,j] ] / (B*N)

No softplus ACT table exists in this compiler, so softplus is computed as
ln(1 + exp(x)) -- exp and ln live in the same ACT table set.  Per-core
pipeline, one [128, 8192] row-block tile per step, engines fully parallel:

  SYNC    dma x row-block -> SBUF                    (~11.7 us/tile)
  ACT     u = exp(x) (bf16), then ln(1+u) with accum_out -> per-row-block
          softplus sums                              (~14.2 us/tile, bound)
  GPSIMD  m = (iota < t) per-partition mask (bf16), independent of x
  DVE     xm = x * m (bf16)
  PE      ones[128,1]^T @ xm chunks accumulate sum(xm) into PSUM [1,512]

Raw Bass with manual semaphores (the Tile framework's exit drain and all
bass_isa raw-ISA ops are rejected by this compiler build).  Host reduces
the tiny outputs in float64.
"""

import numpy as np

_B, _N = 8192, 8192
_NCORES = 8
_ROWS = _B // _NCORES  # 1024 rows per core
_P = 128
_RB = _ROWS // _P  # 8 row-block tiles per core
_MM = 512  # matmul free dim (one PSUM bank)
_NMM = _N // _MM

_cache = {}


def _build_nc(repeat=1):
    import concourse.bass as bass
    import concourse.mybir as mybir

    f32 = mybir.dt.float32
    bf16 = mybir.dt.bfloat16
    A = mybir.AluOpType
    F = mybir.ActivationFunctionType

    nc = bass.Bass()
    x_d = nc.dram_tensor("x", [_ROWS, _N], bf16, kind="ExternalInput")
    tlen_d = nc.dram_tensor("tlen", [_P, _RB], f32, kind="ExternalInput")
    iota_d = nc.dram_tensor("iota", [_P, _N], f32, kind="ExternalInput")
    sp_d = nc.dram_tensor("sp_out", [_P, _RB], f32, kind="ExternalOutput")
    xm_d = nc.dram_tensor("xm_out", [1, _MM], f32, kind="ExternalOutput")

    from contextlib import ExitStack

    with ExitStack() as ctx:
        xt2 = ctx.enter_context(nc.sbuf_tensor([_P, 2 * _N], bf16))  # 2-buf x
        ut2 = ctx.enter_context(nc.sbuf_tensor([_P, 2 * _N], bf16))  # 2-buf exp/ln
        mt2 = ctx.enter_context(nc.sbuf_tensor([_P, 2 * _N], bf16))  # 2-buf mask
        xmt2 = ctx.enter_context(nc.sbuf_tensor([_P, 2 * _N], bf16))  # 2-buf x*m
        iota_f = ctx.enter_context(nc.sbuf_tensor([_P, _N], f32))
        tlen_sb = ctx.enter_context(nc.sbuf_tensor([_P, _RB], f32))
        sp_acc = ctx.enter_context(nc.sbuf_tensor([_P, _RB], f32))
        ones = ctx.enter_context(nc.sbuf_tensor([_P, 1], bf16))
        xm_sb = ctx.enter_context(nc.sbuf_tensor([1, _MM], f32))
        psum_acc = ctx.enter_context(nc.psum_tensor([1, _MM], f32))
        dsem0 = ctx.enter_context(nc.semaphore())  # x loads, even row-blocks
        dsem1 = ctx.enter_context(nc.semaphore())  # x loads, odd row-blocks
        tsem = ctx.enter_context(nc.semaphore())  # tlen load (+16)
        isem = ctx.enter_context(nc.semaphore())  # iota ready
        osem = ctx.enter_context(nc.semaphore())  # ones ready
        asem = ctx.enter_context(nc.semaphore())  # exp completions
        lsem = ctx.enter_context(nc.semaphore())  # ln completions
        gsem = ctx.enter_context(nc.semaphore())  # mask completions
        vsem = ctx.enter_context(nc.semaphore())  # x*m completions
        pesem = ctx.enter_context(nc.semaphore())  # matmul-group completions
        csem = ctx.enter_context(nc.semaphore())  # psum copy done
        fsem = ctx.enter_context(nc.semaphore())  # final out dmas
        block = ctx.enter_context(nc.Block())
        xt = [xt2[:, :_N], xt2[:, _N:]]
        ut = [ut2[:, :_N], ut2[:, _N:]]
        mt = [mt2[:, :_N], mt2[:, _N:]]
        xmt = [xmt2[:, :_N], xmt2[:, _N:]]

        _T = repeat * _RB

        @block.sync
        def _(sync):
            sync.dma_start(out=tlen_sb[:], in_=tlen_d[:]).then_inc(tsem, 16)
            sync.dma_start(out=iota_f[:], in_=iota_d[:]).then_inc(isem, 16)
            for vt in range(_T):
                rb = vt % _RB
                if vt >= 2:
                    sync.wait_ge(asem, vt - 1)
                    sync.wait_ge(vsem, vt - 1)
                sync.dma_start(
                    out=xt[vt % 2], in_=x_d[rb * _P : (rb + 1) * _P, :]
                ).then_inc(dsem0 if vt % 2 == 0 else dsem1, 16)
            # final outputs
            sync.wait_ge(lsem, _T)
            sync.dma_start(out=sp_d[:], in_=sp_acc[:]).then_inc(fsem, 16)
            sync.wait_ge(csem, 1)
            sync.dma_start(out=xm_d[:], in_=xm_sb[:]).then_inc(fsem, 16)
            sync.wait_ge(fsem, 32)

        @block.scalar
        def _(scalar):
            def emit_ln(vt):
                scalar.wait_ge(asem, vt + 1)
                nc.scalar.activation(
                    ut[vt % 2],
                    ut[vt % 2],
                    F.Ln,
                    bias=1.0,
                    scale=1.0,
                    accum_out=sp_acc[:, (vt % _RB) : (vt % _RB) + 1],
                ).then_inc(lsem, 1)

            for vt in range(_T):
                scalar.wait_ge(dsem0 if vt % 2 == 0 else dsem1, 16 * (vt // 2 + 1))
                if vt >= 2:
                    scalar.wait_ge(lsem, vt - 1)  # ut[vt%2] freed by ln vt-2
                nc.scalar.activation(ut[vt % 2], xt[vt % 2], F.Exp).then_inc(asem, 1)
                if vt >= 1:
                    emit_ln(vt - 1)  # ln vt-1 runs behind exp vt: no RAW bubble
            emit_ln(_T - 1)

        @block.vector
        def _(vector):
            nc.vector.memset(ones[:], 1.0).then_inc(osem, 1)
            vector.wait_ge(isem, 16)
            vector.wait_ge(tsem, 16)
            for vt in range(_T):
                rb = vt % _RB
                if vt >= 2:
                    vector.wait_ge(vsem, vt - 1)  # mt[vt%2] freed by tt vt-2
                nc.vector.tensor_scalar(
                    out=mt[vt % 2],
                    in0=iota_f[:],
                    scalar1=tlen_sb[:, rb : rb + 1],
                    scalar2=None,
                    op0=A.is_lt,
                ).then_inc(gsem, 1)
                vector.wait_ge(dsem0 if vt % 2 == 0 else dsem1, 16 * (vt // 2 + 1))
                vector.wait_ge(gsem, vt + 1)
                if vt >= 2:
                    vector.wait_ge(pesem, vt - 1)
                nc.vector.tensor_tensor(
                    out=xmt[vt % 2], in0=xt[vt % 2], in1=mt[vt % 2], op=A.mult
                ).then_inc(vsem, 1)
            vector.wait_ge(pesem, _T)
            nc.vector.tensor_copy(xm_sb[:], psum_acc[:]).then_inc(csem, 1)

        @block.tensor
        def _(tensor):
            tensor.wait_ge(osem, 1)
            for vt in range(_T):
                tensor.wait_ge(vsem, vt + 1)
                for k in range(_NMM):
                    ins = nc.tensor.matmul(
                        psum_acc[:],
                        ones[:],
                        xmt[vt % 2][:, k * _MM : (k + 1) * _MM],
                        start=(vt == 0 and k == 0),
                        stop=(vt == _T - 1 and k == _NMM - 1),
                    )
                ins.then_inc(pesem, 1)

    return nc


def _get_nc():
    if "nc" not in _cache:
        _cache["nc"] = _build_nc()
    return _cache["nc"]


def _prep_in_maps(inputs, targets):
    import ml_dtypes

    x = np.asarray(inputs, dtype=np.float32)
    t = np.asarray(targets).astype(np.float64)  # values < 2**24, exact in f32
    assert x.shape == (_B, _N) and t.shape == (_B,)
    xb = x.astype(ml_dtypes.bfloat16)
    iota = np.ascontiguousarray(
        np.broadcast_to(np.arange(_N, dtype=np.float32)[None, :], (_P, _N))
    )
    in_maps = []
    for c in range(_NCORES):
        xs = np.ascontiguousarray(xb[c * _ROWS : (c + 1) * _ROWS])
        ts = t[c * _ROWS : (c + 1) * _ROWS]
        # tlen[p, rb] = targets[c*1024 + rb*128 + p]
        tlen = np.ascontiguousarray(ts.reshape(_RB, _P).T.astype(np.float32))
        in_maps.append({"x": xs, "tlen": tlen, "iota": iota})
    return in_maps


def kernel(inputs, targets):
    from concourse.bass_utils import run_bass_kernel_spmd

    nc = _get_nc()
    in_maps = _prep_in_maps(inputs, targets)

    res = run_bass_kernel_spmd(nc, in_maps, list(range(_NCORES)))

    total = np.float64(0.0)
    for c in range(_NCORES):
        total += np.sum(res.results[c]["sp_out"].astype(np.float64))
        total -= np.sum(res.results[c]["xm_out"].astype(np.float64))
    loss = total / (np.float64(_B) * np.float64(_N))
    return np.float32(loss)


# revision 16
# speedup vs baseline: 1.0717x; 1.0717x over previous
"""BCEWithLogitsLoss(mean) over (8192, 8192) logits with binary-step targets,
data-parallel over 8 NeuronCores (1024 rows each).

loss = mean(softplus(x) - x * t),  t[i,j] = 1 if j < targets[i] else 0
     = [ sum softplus(x)  -  sum_{j<t_i} x[i,j] ] / (B*N)

No softplus ACT table exists in this compiler, so softplus is computed as
ln(1 + exp(x)) -- exp and ln live in the same ACT table set.  x ships to
the device as bf16 (host-converted), halving HBM traffic (~5e-5 relative
loss error).  Per-core pipeline, one [128, 8192] row-block tile per step:

  SYNC  dma x row-block (2 MiB bf16) -> SBUF; iota/tlen once at start
  ACT   u = exp(x) (bf16), then ln(1+u) with accum_out -> per-row-block
        softplus sums
  DVE   ONE fused op: scalar_tensor_tensor
            out = (iota < t) * x,  accum_out = per-partition sum
        i.e. the whole masked-sum term in a single instruction per tile

Raw Bass with manual semaphores (the Tile framework's exit drain and all
bass_isa raw-ISA ops are rejected by this environment's compiler build).
Host reduces the tiny [128, 8] partial-sum outputs in float64.
"""

import numpy as np

_B, _N = 8192, 8192
_NCORES = 8
_ROWS = _B // _NCORES  # 1024 rows per core
_P = 128
_RB = _ROWS // _P  # 8 row-block tiles per core

_cache = {}


def _build_nc(repeat=1):
    import concourse.bass as bass
    import concourse.mybir as mybir

    f32 = mybir.dt.float32
    bf16 = mybir.dt.bfloat16
    A = mybir.AluOpType
    F = mybir.ActivationFunctionType

    nc = bass.Bass()
    x_d = nc.dram_tensor("x", [_ROWS, _N], bf16, kind="ExternalInput")
    tlen_d = nc.dram_tensor("tlen", [_P, _RB], f32, kind="ExternalInput")
    iota_d = nc.dram_tensor("iota", [_P, _N], f32, kind="ExternalInput")
    sp_d = nc.dram_tensor("sp_out", [_P, _RB], f32, kind="ExternalOutput")
    xma_d = nc.dram_tensor("xma_out", [_P, _RB], f32, kind="ExternalOutput")

    from contextlib import ExitStack

    with ExitStack() as ctx:
        xt2 = ctx.enter_context(nc.sbuf_tensor([_P, 2 * _N], bf16))  # 2-buf x
        ut2 = ctx.enter_context(nc.sbuf_tensor([_P, 2 * _N], bf16))  # 2-buf exp/ln
        jt2 = ctx.enter_context(nc.sbuf_tensor([_P, 2 * _N], bf16))  # 2-buf stt junk
        iota_f = ctx.enter_context(nc.sbuf_tensor([_P, _N], f32))
        tlen_sb = ctx.enter_context(nc.sbuf_tensor([_P, _RB], f32))
        sp_acc = ctx.enter_context(nc.sbuf_tensor([_P, _RB], f32))
        xma_acc = ctx.enter_context(nc.sbuf_tensor([_P, _RB], f32))
        dsem0 = ctx.enter_context(nc.semaphore())  # x loads, even tiles
        dsem1 = ctx.enter_context(nc.semaphore())  # x loads, odd tiles
        tsem = ctx.enter_context(nc.semaphore())  # tlen load (+16)
        isem = ctx.enter_context(nc.semaphore())  # iota load (+16)
        asem = ctx.enter_context(nc.semaphore())  # exp completions
        lsem = ctx.enter_context(nc.semaphore())  # ln completions
        vsem = ctx.enter_context(nc.semaphore())  # stt completions
        fsem = ctx.enter_context(nc.semaphore())  # final out dmas
        block = ctx.enter_context(nc.Block())
        xt = [xt2[:, :_N], xt2[:, _N:]]
        ut = [ut2[:, :_N], ut2[:, _N:]]
        jt = [jt2[:, :_N], jt2[:, _N:]]

        _T = repeat * _RB

        @block.sync
        def _(sync):
            sync.dma_start(out=tlen_sb[:], in_=tlen_d[:]).then_inc(tsem, 16)
            sync.dma_start(out=iota_f[:], in_=iota_d[:]).then_inc(isem, 16)
            for vt in range(_T):
                rb = vt % _RB
                if vt >= 2:
                    sync.wait_ge(asem, vt - 1)
                    sync.wait_ge(vsem, vt - 1)
                sync.dma_start(
                    out=xt[vt % 2], in_=x_d[rb * _P : (rb + 1) * _P, :]
                ).then_inc(dsem0 if vt % 2 == 0 else dsem1, 16)
            # final outputs
            sync.wait_ge(lsem, _T)
            sync.dma_start(out=sp_d[:], in_=sp_acc[:]).then_inc(fsem, 16)
            sync.wait_ge(vsem, _T)
            sync.dma_start(out=xma_d[:], in_=xma_acc[:]).then_inc(fsem, 16)
            sync.wait_ge(fsem, 32)

        @block.scalar
        def _(scalar):
            def emit_ln(vt):
                scalar.wait_ge(asem, vt + 1)
                nc.scalar.activation(
                    ut[vt % 2],
                    ut[vt % 2],
                    F.Ln,
                    bias=1.0,
                    scale=1.0,
                    accum_out=sp_acc[:, (vt % _RB) : (vt % _RB) + 1],
                ).then_inc(lsem, 1)

            for vt in range(_T):
                scalar.wait_ge(dsem0 if vt % 2 == 0 else dsem1, 16 * (vt // 2 + 1))
                if vt >= 2:
                    scalar.wait_ge(lsem, vt - 1)  # ut[vt%2] freed by ln vt-2
                nc.scalar.activation(ut[vt % 2], xt[vt % 2], F.Exp).then_inc(asem, 1)
                if vt >= 1:
                    emit_ln(vt - 1)  # ln vt-1 runs behind exp vt: no RAW bubble
            emit_ln(_T - 1)

        @block.vector
        def _(vector):
            vector.wait_ge(isem, 16)
            vector.wait_ge(tsem, 16)
            for vt in range(_T):
                rb = vt % _RB
                vector.wait_ge(dsem0 if vt % 2 == 0 else dsem1, 16 * (vt // 2 + 1))
                if vt >= 2:
                    vector.wait_ge(vsem, vt - 1)  # jt[vt%2] freed by stt vt-2
                nc.vector.scalar_tensor_tensor(
                    out=jt[vt % 2],
                    in0=iota_f[:],
                    scalar=tlen_sb[:, rb : rb + 1],
                    in1=xt[vt % 2],
                    op0=A.is_lt,
                    op1=A.mult,
                    accum_out=xma_acc[:, rb : rb + 1],
                ).then_inc(vsem, 1)

    return nc


def _get_nc():
    if "nc" not in _cache:
        _cache["nc"] = _build_nc()
    return _cache["nc"]


def _prep_in_maps(inputs, targets):
    import ml_dtypes

    x = np.asarray(inputs, dtype=np.float32)
    t = np.asarray(targets).astype(np.float64)  # values < 2**24, exact in f32
    assert x.shape == (_B, _N) and t.shape == (_B,)
    xb = x.astype(ml_dtypes.bfloat16)
    iota = np.ascontiguousarray(
        np.broadcast_to(np.arange(_N, dtype=np.float32)[None, :], (_P, _N))
    )
    in_maps = []
    for c in range(_NCORES):
        xs = np.ascontiguousarray(xb[c * _ROWS : (c + 1) * _ROWS])
        ts = t[c * _ROWS : (c + 1) * _ROWS]
        # tlen[p, rb] = targets[c*1024 + rb*128 + p]
        tlen = np.ascontiguousarray(ts.reshape(_RB, _P).T.astype(np.float32))
        in_maps.append({"x": xs, "tlen": tlen, "iota": iota})
    return in_maps


def kernel(inputs, targets):
    from concourse.bass_utils import run_bass_kernel_spmd

    nc = _get_nc()
    in_maps = _prep_in_maps(inputs, targets)

    res = run_bass_kernel_spmd(nc, in_maps, list(range(_NCORES)))

    total = np.float64(0.0)
    for c in range(_NCORES):
        total += np.sum(res.results[c]["sp_out"].astype(np.float64))
        total -= np.sum(res.results[c]["xma_out"].astype(np.float64))
    loss = total / (np.float64(_B) * np.float64(_N))
    return np.float32(loss)
